# revision 26
# baseline (speedup 1.0000x reference)
"""Trainium2 Bass kernel for nn_CrossAttention (B=4, S=2048, 16 heads x 64).

Sharding: 8 cores = 4 batches x 2 head-groups (8 heads each).
Per-core device program (all layouts chosen so no on-device transposes needed):
  QT[e,t] = Wq_g^T x_q^T   (channels on partitions)   via lhsT=Wq_g, rhs=xqT
  KT[e,t] likewise; V[t,e] natural layout via lhsT=xvT, rhs=Wv_g
  Per head h, q-chunk: S^T[k,q] psum = KT_h^T QT_h;  E = exp(S^T/8) (ACT, PSUM->SBUF)
  PV with a ones-column appended to V: psum_o[0:64]=V_h^T E (unnorm O^T),
  psum_o[64] = colsum(E) = softmax denominators.
  recip -> outer-product matmul broadcasts 1/denom to 128 partitions ->
  normalize E (-> attn prob output, written transposed; host untransposes)
  and O^T; Wo projection of normalized O^T gives the per-group partial output.
Host: sums the two head-group partials per batch, adds bv@Wo + bo (exact since
softmax rows sum to 1), transposes prob tiles back to [q,k].
"""

import os
import sys

sys.path.insert(0, "/opt/trn_rl_repo")

import numpy as np

import concourse.bass as bass
import concourse.tile as tile
from concourse import bacc, mybir
from concourse import bass_utils

F32 = mybir.dt.float32
F32R = mybir.dt.float32r
BF16 = mybir.dt.bfloat16
import os as _os
P_BF16 = _os.environ.get("KERNEL_P_F32", "0") != "1"
P_DT = BF16 if P_BF16 else F32R
AF = mybir.ActivationFunctionType

# problem dims (hardcoded per contract)
B = 4
S = 2048          # sequence length
QD, KD, VD = 1024, 768, 768
E_OUT = 1024      # output embed
H_ALL, D = 16, 64
G = 2             # head groups (cores per batch)
HG = H_ALL // G   # heads per core = 8
EG = HG * D       # group channels = 512
SCALE = D ** -0.5

TQ = 512          # token chunk (matmul free dim)
NKT = S // 128    # 16 key-token tiles
NQC = S // TQ     # 4 q chunks
GPS_KT = 5        # of 16 normalize multiplies, how many go to GPSIMD

_NC_CACHE = {}
LAST_RESULT = None


def build_nc():
    nc = bacc.Bacc(None, target_bir_lowering=False)

    xqT = nc.dram_tensor("xqT", [QD, S], F32R, kind="ExternalInput")
    xkT = nc.dram_tensor("xkT", [KD, S], F32R, kind="ExternalInput")
    xvT = nc.dram_tensor("xvT", [VD, S], F32R, kind="ExternalInput")
    wq = nc.dram_tensor("wq", [QD, EG], F32R, kind="ExternalInput")
    wk = nc.dram_tensor("wk", [KD, EG], F32R, kind="ExternalInput")
    wv = nc.dram_tensor("wv", [VD, EG], F32R, kind="ExternalInput")
    wo = nc.dram_tensor("wo", [EG, E_OUT], F32R, kind="ExternalInput")
    bqp = nc.dram_tensor("bqp", [128, EG // 128], F32, kind="ExternalInput")
    bkp = nc.dram_tensor("bkp", [128, EG // 128], F32, kind="ExternalInput")
    p_out = nc.dram_tensor("p_out", [HG, S, S], P_DT, kind="ExternalOutput")
    o_out = nc.dram_tensor("o_out", [S, E_OUT], F32, kind="ExternalOutput")

    NMQ = EG // 128   # 4 channel tiles of the group embed
    NKQ = QD // 128   # 8 contraction tiles for Q proj
    NKK = KD // 128   # 6 for K/V proj

    with tile.TileContext(nc) as tc:
        with (
            tc.tile_pool(name="persist", bufs=1) as persist,
        ):
            QT = persist.tile([128, NMQ, S], F32R)    # [chan%128, chan//128, tok]
            KT = persist.tile([128, NMQ, S], F32R)
            Vp = persist.tile([128, NKT, HG, D + 1],
                              BF16 if P_BF16 else F32R)  # V + ones col
            ones_sb = persist.tile([128, 128], F32R)
            bq_sb = persist.tile([128, NMQ], F32)
            bk_sb = persist.tile([128, NMQ], F32)

            ones_f32 = persist.tile([128, 128], F32)
            nc.vector.memset(ones_f32[:], 1.0)
            nc.vector.tensor_copy(ones_sb[:], ones_f32[:])
            nc.vector.tensor_copy(
                Vp[:, :, :, D],
                ones_f32.rearrange("p (a b) -> p a b", a=NKT))
            nc.sync.dma_start(bq_sb[:], bqp[:])
            nc.sync.dma_start(bk_sb[:], bkp[:])

            # ---------------- phase 1: projections ----------------
            with (
                tc.tile_pool(name="projw", bufs=1) as projw,
                tc.tile_pool(name="projx", bufs=2) as projx,
                tc.tile_pool(name="projps", bufs=4, space="PSUM") as projps,
            ):
                xqT_r = xqT.rearrange("(ko p) t -> p ko t", p=128)
                xkT_r = xkT.rearrange("(ko p) t -> p ko t", p=128)
                xvT_r = xvT.rearrange("(ko p) t -> p ko t", p=128)

                wq_sb = projw.tile([128, NKQ, EG], F32R)
                nc.sync.dma_start(wq_sb[:], wq.rearrange("(ko p) m -> p ko m", p=128))
                xq_first = projx.tile(
                    [128, NKQ, TQ], F32R, tag="xch", name="xq_first")
                nc.sync.dma_start(xq_first[:], xqT_r[:, :, 0:TQ])
                wk_sb = projw.tile([128, NKK, EG], F32R)
                nc.sync.dma_start(wk_sb[:], wk.rearrange("(ko p) m -> p ko m", p=128))
                wv_sb = projw.tile([128, NKK, EG], F32R)
                nc.sync.dma_start(wv_sb[:], wv.rearrange("(ko p) m -> p ko m", p=128))

                for nj in range(NQC):
                    ts = slice(nj * TQ, (nj + 1) * TQ)
                    if nj == 0:
                        xq_ch = xq_first
                    else:
                        xq_ch = projx.tile([128, NKQ, TQ], F32R, tag="xch")
                        nc.sync.dma_start(xq_ch[:], xqT_r[:, :, ts])
                    for mi in range(NMQ):
                        ps = projps.tile([128, TQ], F32, tag="pps")
                        for ki in range(NKQ):
                            nc.tensor.matmul(
                                ps[:],
                                (wq_sb[:, ki, mi * 128:(mi + 1) * 128]),
                                (xq_ch[:, ki, :]),
                                start=(ki == 0), stop=(ki == NKQ - 1),
                            )
                        nc.vector.tensor_scalar_add(
                            QT[:, mi, ts], ps[:], bq_sb[:, mi:mi + 1])

                for nj in range(NQC):
                    ts = slice(nj * TQ, (nj + 1) * TQ)
                    xk_ch = projx.tile(
                        [128, NKQ, TQ], F32R, tag="xch", name="xk_ch")[:, :NKK, :]
                    nc.sync.dma_start(xk_ch[:], xkT_r[:, :, ts])
                    for mi in range(NMQ):
                        ps = projps.tile([128, TQ], F32, tag="pps")
                        for ki in range(NKK):
                            nc.tensor.matmul(
                                ps[:],
                                (wk_sb[:, ki, mi * 128:(mi + 1) * 128]),
                                (xk_ch[:, ki, :]),
                                start=(ki == 0), stop=(ki == NKK - 1),
                            )
                        nc.vector.tensor_scalar_add(
                            KT[:, mi, ts], ps[:], bk_sb[:, mi:mi + 1])

                for nj in range(NQC):
                    ts = slice(nj * TQ, (nj + 1) * TQ)
                    xv_ch = projx.tile(
                        [128, NKQ, TQ], F32R, tag="xch", name="xv_ch")[:, :NKK, :]
                    nc.sync.dma_start(xv_ch[:], xvT_r[:, :, ts])
                    for tj in range(TQ // 128):
                        kt = nj * (TQ // 128) + tj
                        ps = projps.tile([128, EG], F32, tag="pps")
                        for ki in range(NKK):
                            nc.tensor.matmul(
                                ps[:],
                                (xv_ch[:, ki, tj * 128:(tj + 1) * 128]),
                                (wv_sb[:, ki, :]),
                                start=(ki == 0), stop=(ki == NKK - 1),
                            )
                        nc.vector.tensor_copy(
                            Vp[:, kt, :, 0:D],
                            ps.rearrange("p (h d) -> p h d", h=HG))

            # ------------- phase 2+3: attention + output projection -------------
            ADT = BF16 if P_BF16 else F32R
            NKD = EG // 128  # 4 d-tiles for Wo contraction
            with (
                tc.tile_pool(name="wop", bufs=1) as wop,
                tc.tile_pool(name="attn_sb", bufs=1) as asb,
                tc.tile_pool(name="attn_ps", bufs=1, space="PSUM") as aps,
            ):
                wo_sb = wop.tile([128, NKD, E_OUT], F32R)
                nc.sync.dma_start(wo_sb[:], wo.rearrange("(kd p) e -> p kd e", p=128))

                def dense(qc, h, half):
                    """scores+exp+PV for head h, kt groups half*4..half*4+3."""
                    qs = slice(qc * TQ, (qc + 1) * TQ)
                    hb = D * (h % 2)
                    ht = h // 2
                    st = dense.state
                    if half == 0:
                        st["E"] = asb.tile([128, NKT, TQ], ADT, tag="E",
                                           bufs=3, name="E")
                        st["po"] = aps.tile([D + 1, TQ], F32, tag="po",
                                            bufs=2, name="po")
                    E, po = st["E"], st["po"]
                    for ktg in range(half * 4, half * 4 + 4):
                        ps_s = aps.tile([128, 2 * TQ], F32, tag="s", bufs=2)
                        for j in range(2):
                            kt = 2 * ktg + j
                            nc.tensor.matmul(
                                ps_s[:, j * TQ:(j + 1) * TQ],
                                (KT[hb:hb + D, ht, kt * 128:(kt + 1) * 128]),
                                (QT[hb:hb + D, ht, qs]),
                                start=True, stop=True,
                            )
                        nc.scalar.activation(
                            E[:, 2 * ktg:2 * ktg + 2, :],
                            ps_s.rearrange("p (a b) -> p a b", a=2),
                            AF.Exp, scale=SCALE)
                        for j in range(2):
                            kt = 2 * ktg + j
                            nc.tensor.matmul(
                                po[:],
                                (Vp[:, kt, h, :]),
                                (E[:, kt, :]),
                                start=(kt == 0), stop=(kt == NKT - 1),
                            )
                    if dense.pending:
                        dense.pending.pop(0)()
                    return dict(st) if half == 1 else None
                dense.state = {}
                dense.pending = []

                def norm_a(st):
                    # denominator reciprocal chain (DVE) — runs while the
                    # next head's dense phase occupies PE/ACT
                    po = st["po"]
                    osb = asb.tile([D + 1, TQ], F32R, tag="osb", bufs=2,
                                   name="osb")
                    nc.vector.tensor_copy(osb[:], po[:])
                    with nc.allow_low_precision(reason="f32r softmax denom"):
                        nc.vector.reciprocal(osb[D:D + 1, :], osb[D:D + 1, :])
                    st["osb"] = osb

                def norm_b(st):
                    # broadcast 1/denom across partitions (PE outer product)
                    osb = st["osb"]
                    pb = aps.tile([128, TQ], F32, tag="pb", bufs=1, name="pb")
                    nc.tensor.matmul(
                        pb[:],
                        (ones_sb[D:D + 1, :]),
                        (osb[D:D + 1, :]),
                        start=True, stop=True,
                    )
                    st["pb"] = pb

                def norm_c(st):
                    # normalize O^T + P tiles, store out
                    h, qc = st["h"], st["qc"]
                    OT_sb = st["OT"]
                    ht = h // 2
                    qs = slice(qc * TQ, (qc + 1) * TQ)
                    E, po, osb, pb = st["E"], st["po"], st["osb"], st["pb"]
                    bsb = asb.tile([128, TQ], ADT, tag="bsb", bufs=2,
                                   name="bsb")
                    nc.vector.tensor_copy(bsb[:], pb[:])
                    if h % 2 == 0:
                        nc.vector.tensor_mul(
                            OT_sb[0:D, ht, :], osb[0:D, :], pb[0:D, :])
                        st["ot_dma"] = None
                    else:
                        nc.vector.tensor_mul(
                            osb[0:D, :], osb[0:D, :], pb[0:D, :])
                            # placed into partitions 64..127 via DMA below
                        nc.sync.dma_start(
                            OT_sb[D:2 * D, ht, :], osb[0:D, :])
                    P = asb.tile([128, NKT, TQ], ADT, tag="P", bufs=2,
                                 name="P")
                    p_dst = p_out[h, :, qs].rearrange(
                        "(kt p) q -> p kt q", p=128)
                    for kt in range(NKT):
                        if kt % 4 == 1:
                            nc.gpsimd.tensor_mul(
                                P[:, kt, :], E[:, kt, :], bsb[:])
                        else:
                            nc.vector.tensor_mul(
                                P[:, kt, :], E[:, kt, :], bsb[:])
                        if kt % 4 == 3:
                            nc.sync.dma_start(
                                p_dst[:, kt - 3:kt + 1, :],
                                P[:, kt - 3:kt + 1, :])

                prev = None
                for qc in range(NQC):
                    OT_sb = asb.tile([128, NKD, TQ], F32R, tag="OT",
                                     bufs=2, name="OT")
                    dense.state["OT"] = OT_sb
                    for h in range(HG):
                        dense(qc, h, 0)
                        if prev is not None:
                            norm_b(prev)
                        st = dense(qc, h, 1)
                        st["h"], st["qc"] = h, qc
                        if prev is not None:
                            norm_c(prev)
                        norm_a(st)
                        prev = st
                    # flush the last head before this qc's output projection
                    norm_b(prev)
                    norm_c(prev)
                    prev = None
                    # ---- output projection for this qc (deferred: drained
                    # one psum-group per dense-half of the next qc) ----
                    def push_p3(qc, OT_sb=None):
                        OT_sb = OT_sb if OT_sb is not None else dense.state["OT"]
                        def mk(mql, n2):
                            def emit():
                                mq = qc * (TQ // 128) + mql
                                ps = aps.tile([128, TQ], F32, tag="wps",
                                              bufs=1, name="wps")
                                for kd in range(NKD):
                                    nc.tensor.matmul(
                                        ps[:],
                                        (OT_sb[:, kd,
                                               mql * 128:(mql + 1) * 128]),
                                        (wo_sb[:, kd,
                                               n2 * TQ:(n2 + 1) * TQ]),
                                        start=(kd == 0),
                                        stop=(kd == NKD - 1),
                                    )
                                ob = asb.tile([128, TQ], F32, tag="ob",
                                              bufs=2, name="ob")
                                nc.vector.tensor_copy(ob[:], ps[:])
                                nc.sync.dma_start(
                                    o_out[mq * 128:(mq + 1) * 128,
                                          n2 * TQ:(n2 + 1) * TQ], ob[:])
                            return emit
                        for mql in range(TQ // 128):
                            for n2 in range(E_OUT // TQ):
                                dense.pending.append(mk(mql, n2))
                    push_p3(qc)
                # end of all qc: drain any remaining projection groups
                while dense.pending:
                    dense.pending.pop(0)()

    nc.compile()
    return nc


def kernel(**inputs):
    global LAST_RESULT
    q = np.asarray(inputs["query"], dtype=np.float32)
    k = np.asarray(inputs["key"], dtype=np.float32)
    v = np.asarray(inputs["value"], dtype=np.float32)
    Wq = np.asarray(inputs["Wq"], dtype=np.float32)
    Wk = np.asarray(inputs["Wk"], dtype=np.float32)
    Wv = np.asarray(inputs["Wv"], dtype=np.float32)
    Wo = np.asarray(inputs["Wo"], dtype=np.float32)
    bq = np.asarray(inputs["bq"], dtype=np.float32)
    bk = np.asarray(inputs["bk"], dtype=np.float32)
    bv = np.asarray(inputs["bv"], dtype=np.float32)
    bo = np.asarray(inputs["bo"], dtype=np.float32)

    if "nc" not in _NC_CACHE:
        _NC_CACHE["nc"] = build_nc()
    nc = _NC_CACHE["nc"]

    in_maps = []
    for c in range(8):
        b, g = divmod(c, G)
        gs = slice(g * EG, (g + 1) * EG)
        in_maps.append({
            "xqT": np.ascontiguousarray(q[b].T),
            "xkT": np.ascontiguousarray(k[b].T),
            "xvT": np.ascontiguousarray(v[b].T),
            "wq": np.ascontiguousarray(Wq[:, gs]),
            "wk": np.ascontiguousarray(Wk[:, gs]),
            "wv": np.ascontiguousarray(Wv[:, gs]),
            "wo": np.ascontiguousarray(Wo[gs, :]),
            "bqp": np.ascontiguousarray(bq[gs].reshape(EG // 128, 128).T),
            "bkp": np.ascontiguousarray(bk[gs].reshape(EG // 128, 128).T),
        })

    trace = bool(int(os.environ.get("KERNEL_TRACE", "0")))
    res = bass_utils.run_bass_kernel_spmd(
        nc, in_maps, core_ids=list(range(8)), trace=trace)
    LAST_RESULT = res

    attn_output = np.zeros((B, S, E_OUT), dtype=np.float32)
    attn_weights = np.empty((B, H_ALL, S, S), dtype=np.float32)
    for c in range(8):
        b, g = divmod(c, G)
        r = res.results[c]
        attn_output[b] += r["o_out"]
        pT = r["p_out"]  # [HG, S(k), S(q)]
        if pT.dtype != np.float32:
            pT = pT.astype(np.float32)
        for h in range(HG):
            attn_weights[b, g * HG + h] = pT[h].T
    attn_output += bv @ Wo + bo
    return attn_output, attn_weights
